# revision 12
# baseline (speedup 1.0000x reference)
"""Trainium2 Bass kernel for nn_KnotEntangle (K=1024, SAMPLES=4096, 8 cores).

Math: the FFT collapses — signal[:,0] = rowsum(smear) and sum_sig = S*smear[:,0].
The [K,K]@[K,S] contraction reduces to result = w @ smear with
w = coef1 + coef2 + c3 built from row/col reductions of the mix matrix
(rank-1 corr => PE-computable deltas). Each gaussian is evaluated as
(sqrt(pi)/2)*Derivative_Erf(u), u = min(max(d*(-a), d*b), CLAMP) via one
custom DVE op; deltas come from rank-2 PE matmuls.

Sharding: knots split 128/core across 8 cores; [K] summaries AllGathered;
[2,4096] partial (result,env) rows AllReduced; output = env*result (each
core holds the full product, so the output is fetched replicated).

Dispatch: the axon tunnel has ~84ms RTT, which dominates wall-clock. The
jitted shard_map callable is built once and cached; per call we upload one
packed input tensor per core, run, and fetch one 16KB replicated output
with a single blocking point so upload+exec+fetch pipeline into ~1 RTT.
"""
import numpy as np

import jax
from jax.sharding import Mesh, PartitionSpec, NamedSharding

import concourse.bacc as bacc
import concourse.tile as tile
import concourse.mybir as mybir

import concourse.dve_ops as dve_ops_mod
from concourse.dve_ops import DveOp, TENSOR_TENSOR_REDUCE
from concourse.dve_spec import Spec, Src0, C0, C1, C2, maxx, minn, lower as dve_lower
from concourse.dve_uop import DveOpSpec

K = 1024
SAMPLES = 4096
S = SAMPLES - 1           # 4095
M = 8                     # cores
KL = K // M               # 128 knots per core
SQ = float(np.sqrt(np.pi) / 2.0)
CLAMP = 30.0
CH = [(512 * i, 512) for i in range(7)] + [(3584, 511)]   # s-chunks

# packed per-core input layout (float32 offsets)
O_UNIQ = 0                 # 14 slots x KL: xs xm es eo nar brr nae bee
N_UNIQ = 14 * KL           #   naentL bentL emloc nemloc cospL sinpL
O_N8 = N_UNIQ              # naent8 [KL,M]
O_B8 = O_N8 + KL * M       # bent8  [KL,M]
O_C8 = O_B8 + KL * M       # cosp8  [KL,M]
O_NEM = O_C8 + KL * M      # nem_full [K]
PB = O_NEM + K             # 5888 floats per core


def _selmax_ref(in0, in1, s0, s1, imm2):
    return np.minimum(np.maximum(in0 * s0, in0 * s1), imm2).astype(np.float32)


def _make_selmax_op():
    name = "KNOT_SELMAX"
    if name in dve_ops_mod._SUB_OPCODE_FOR_NAME:
        return next(op for op in dve_ops_mod.OPS if op.name == name)
    spec = Spec(body=minn(maxx(Src0 * C0, Src0 * C1), C2), reference=_selmax_ref)
    row = dve_ops_mod._CUSTOM_DVE_ROW_BASE + len(dve_ops_mod.OPS)
    assert row < 0x20
    dve_ops_mod._SUB_OPCODE_FOR_NAME[name] = row
    shas = {}
    for ver in ("v3", "v4"):
        uops = dve_lower(spec, ver=ver)
        shas[ver] = DveOpSpec(name=name, opcode=row, uops=uops, rd1_en=False).sha(ver)
    op = DveOp(name, spec, subdim=False, uops_sha=shas)
    dve_ops_mod.OPS.append(op)
    dve_ops_mod.CUSTOM_DVE_SPECS[name] = spec
    return op


_CACHE = {}


def _build():
    if "nc" in _CACHE:
        return _CACHE["nc"]
    SELMAX = _make_selmax_op()
    nc = bacc.Bacc(None, target_bir_lowering=False, num_devices=M)
    f32 = mybir.dt.float32
    AF = mybir.ActivationFunctionType

    packed_d = nc.dram_tensor("packed", [PB], f32, kind="ExternalInput")
    out_d = nc.dram_tensor("out", [S], f32, kind="ExternalOutput")

    agin = nc.dram_tensor("agin", [2 * KL], f32, kind="Internal")
    agout = nc.dram_tensor("agout", [M * 2 * KL], f32, kind="Internal",
                           addr_space="Shared")
    arin = nc.dram_tensor("arin", [2, SAMPLES], f32, kind="Internal")
    arout = nc.dram_tensor("arout", [2, SAMPLES], f32, kind="Internal",
                           addr_space="Shared")
    coefd = nc.dram_tensor("coefd", [2 * KL], f32, kind="Internal")
    rg = [list(range(M))]

    with tile.TileContext(nc) as tc:
        with tc.tile_pool(name="big", bufs=1) as big, \
             tc.tile_pool(name="sml", bufs=1) as sml, \
             tc.tile_pool(name="u", bufs=3) as upool, \
             tc.tile_pool(name="acc", bufs=8) as accp, \
             tc.tile_pool(name="pd", bufs=3, space="PSUM") as pd, \
             tc.tile_pool(name="pr", bufs=1, space="PSUM") as pr, \
             tc.tile_pool(name="pc", bufs=1, space="PSUM") as pc:

            # ---- load constants (slices of the packed input) ----
            # basis row0 = 0..S-1 (exact in f32; 1/SAMPLES folded into xs/es
            # host-side), row1 = ones
            basis = sml.tile([2, S], f32)
            nc.vector.memset(basis[:], 1.0)
            nc.gpsimd.iota(basis[0:1, :], [[1, S]], base=0,
                           channel_multiplier=0,
                           allow_small_or_imprecise_dtypes=True)
            smear_lhsT = sml.tile([2, KL], f32)
            nc.sync.dma_start(smear_lhsT[:],
                              packed_d[0:2 * KL].rearrange("(a b) -> a b", a=2))
            env_lhsT = sml.tile([2, KL], f32)
            nc.sync.dma_start(env_lhsT[:],
                              packed_d[2 * KL:4 * KL].rearrange("(a b) -> a b", a=2))

            def col(slot, name):
                t = sml.tile([KL, 1], f32, tag=f"col_{name}")
                off = slot * KL
                nc.sync.dma_start(t[:], packed_d[off:off + KL][:, None])
                return t

            nar = col(4, "nar"); br = col(5, "brr")
            nae = col(6, "nae"); be = col(7, "bee")
            naentL = col(8, "naentL"); bentL = col(9, "bentL")
            emloc = col(10, "emloc")
            cospL = col(12, "cospL"); sinpL = col(13, "sinpL")
            naent8 = sml.tile([KL, M], f32)
            nc.sync.dma_start(naent8[:],
                              packed_d[O_N8:O_N8 + KL * M]
                              .rearrange("(k m) -> k m", m=M))
            bent8 = sml.tile([KL, M], f32)
            nc.sync.dma_start(bent8[:],
                              packed_d[O_B8:O_B8 + KL * M]
                              .rearrange("(k m) -> k m", m=M))
            cosp8 = sml.tile([KL, M], f32)
            nc.sync.dma_start(cosp8[:],
                              packed_d[O_C8:O_C8 + KL * M]
                              .rearrange("(k m) -> k m", m=M))

            zero_col = sml.tile([KL, 1], f32)
            nc.vector.memset(zero_col[:], 0.0)
            ones_col = sml.tile([KL, 1], f32)
            nc.vector.memset(ones_col[:], 1.0)

            SM = big.tile([KL, S], f32)
            GA = big.tile([KL, S], f32)
            MXa = big.tile([KL, K], f32)
            MXb = big.tile([KL, K], f32)
            ssr_bc = big.tile([KL, K], f32)

            # ---- phase 2: smear ----
            acc8 = accp.tile([KL, M], f32, tag="acc8")
            for ci, (c0, n) in enumerate(CH):
                dl = pd.tile([KL, 512], f32, tag="delta")
                nc.tensor.matmul(dl[:, 0:n], smear_lhsT[:], basis[:, c0:c0 + n],
                                 start=True, stop=True)
                ut = upool.tile([KL, 512], f32, tag="u")
                nc.vector._custom_dve(SELMAX, out=ut[:, 0:n], in0=dl[:, 0:n],
                                      s0=nar[:], s1=br[:], imm2=CLAMP)
                nc.scalar.activation(SM[:, c0:c0 + n], ut[:, 0:n], AF.Derivative_Erf,
                                     bias=zero_col[:], accum_out=acc8[:, ci:ci + 1])
            ssr_p = sml.tile([KL, 1], f32)
            nc.vector.tensor_scalar_mul(ssr_p[:], SM[:, 0:1], float(S) * SQ)
            sig_sum = sml.tile([KL, 1], f32)
            nc.vector.reduce_sum(sig_sum[:], acc8[:], axis=mybir.AxisListType.X)
            sig0p = sml.tile([KL, 1], f32)
            nc.vector.tensor_scalar_mul(sig0p[:], sig_sum[:],
                                        float(SQ / np.sqrt(S)))

            # ---- phase 3: AllGather [sig0p | ssr] ----
            nc.sync.dma_start(agin[0:KL, None], sig0p[:])
            nc.sync.dma_start(agin[KL:2 * KL, None], ssr_p[:])
            nc.gpsimd.collective_compute(
                "AllGather", mybir.AluOpType.bypass, replica_groups=rg,
                ins=[agin[:]], outs=[agout[:]])

            # ---- phase 4: env (overlaps AG) ----
            for (c0, n) in CH:
                dl = pd.tile([KL, 512], f32, tag="delta")
                nc.tensor.matmul(dl[:, 0:n], env_lhsT[:], basis[:, c0:c0 + n],
                                 start=True, stop=True)
                ut = upool.tile([KL, 512], f32, tag="u")
                nc.vector._custom_dve(SELMAX, out=ut[:, 0:n], in0=dl[:, 0:n],
                                      s0=nae[:], s1=be[:], imm2=CLAMP)
                nc.scalar.activation(GA[:, c0:c0 + n], ut[:, 0:n], AF.Derivative_Erf,
                                     bias=zero_col[:])
            # env reduction: two [1,2048] psum halves sharing one pr slot
            env_row = sml.tile([1, S], f32)
            for h in range(2):
                red = pr.tile([1, 2048], f32, tag="red")
                base = 2048 * h
                nv = 2048 if h == 0 else S - 2048
                for (c0, n) in CH[4 * h:4 * h + 4]:
                    nc.tensor.matmul(red[0:1, c0 - base:c0 - base + n], ones_col[:],
                                     GA[:, c0:c0 + n], start=True, stop=True)
                nc.scalar.copy(env_row[0:1, base:base + nv], red[0:1, 0:nv])
            nc.sync.dma_start(arin[1, 0:S][None, :], env_row[:])

            # ---- phase 5: post-AG assembly ----
            rhs_b = sml.tile([2, K], f32)
            nc.vector.memset(rhs_b[:], 1.0)        # row 1 stays all-ones
            mixa_lhsT = sml.tile([2, K], f32)
            ssr8 = sml.tile([KL, M], f32)
            for r in range(M):
                nc.sync.dma_start(rhs_b[0:1, KL * r:KL * (r + 1)],
                                  agout[2 * KL * r:2 * KL * r + KL][None, :])
                nc.sync.dma_start(mixa_lhsT[0:1, KL * r:KL * (r + 1)],
                                  agout[2 * KL * r:2 * KL * r + KL][None, :])
                nc.sync.dma_start(ssr8[:, r:r + 1],
                                  agout[2 * KL * r + KL:2 * KL * (r + 1)][:, None])
                nc.sync.dma_start(
                    ssr_bc[:, KL * r:KL * (r + 1)],
                    agout[2 * KL * r + KL:2 * KL * (r + 1)][None, :]
                    .broadcast_to((KL, KL)))
            nc.sync.dma_start(mixa_lhsT[1:2, :],
                              packed_d[O_NEM:O_NEM + K][None, :])
            rhs_a = sml.tile([2, KL], f32)
            nc.vector.memset(rhs_a[:], 1.0)        # row 1 stays all-ones
            nc.sync.dma_start(rhs_a[0:1, :], agin[0:KL][None, :])
            mixb_lhsT = sml.tile([2, KL], f32)
            nc.sync.dma_start(mixb_lhsT[0:1, :], agin[0:KL][None, :])
            nc.sync.dma_start(mixb_lhsT[1:2, :],
                              packed_d[11 * KL:12 * KL][None, :])
            cw8 = sml.tile([KL, M], f32)
            nc.vector.tensor_tensor(cw8[:], cosp8[:], ssr8[:],
                                    op=mybir.AluOpType.mult)
            wgt = sml.tile([KL, 2 * M], f32)
            nc.vector.memset(wgt[:], 1.0)
            for t in range(M):
                nc.vector.tensor_copy(wgt[:, 2 * t:2 * t + 1], cw8[:, t:t + 1])

            # ---- phase 6: mix block b (cc over global i) ----
            cch = []
            for ci, c0 in enumerate((0, 512)):
                dl = pd.tile([KL, 512], f32, tag="delta")
                nc.tensor.matmul(dl[:], mixb_lhsT[:], rhs_b[:, c0:c0 + 512],
                                 start=True, stop=True)
                ut = upool.tile([KL, 512], f32, tag="u")
                nc.vector._custom_dve(SELMAX, out=ut[:], in0=dl[:],
                                      s0=naentL[:], s1=bentL[:], imm2=CLAMP)
                nc.scalar.activation(MXb[:, c0:c0 + 512], ut[:], AF.Derivative_Erf,
                                     bias=zero_col[:])
                acc = accp.tile([KL, 1], f32, tag="cch")
                trash = upool.tile([KL, 512], f32, tag="u")
                nc.vector._custom_dve(TENSOR_TENSOR_REDUCE, out=trash[:],
                                      in0=MXb[:, c0:c0 + 512],
                                      in1=ssr_bc[:, c0:c0 + 512],
                                      s0=(0.0 if ci == 0 else cch[0][:]), s1=1.0,
                                      accum_out=acc[:])
                cch.append(acc)
            cchat = cch[1]

            # ---- phase 7: mix block a + coef reductions ----
            for t in range(M):
                dl = pd.tile([KL, KL], f32, tag="delta")
                nc.tensor.matmul(dl[:], mixa_lhsT[:, KL * t:KL * (t + 1)], rhs_a[:],
                                 start=True, stop=True)
                ut = upool.tile([KL, KL], f32, tag="u")
                nc.vector._custom_dve(SELMAX, out=ut[:], in0=dl[:],
                                      s0=naent8[:, t:t + 1], s1=bent8[:, t:t + 1],
                                      imm2=CLAMP)
                nc.scalar.activation(MXa[:, KL * t:KL * (t + 1)], ut[:],
                                     AF.Derivative_Erf, bias=zero_col[:])
            coef_ps = pc.tile([2, KL], f32)
            for t in range(M):
                nc.tensor.matmul(coef_ps[:], wgt[:, 2 * t:2 * t + 2],
                                 MXa[:, KL * t:KL * (t + 1)],
                                 start=(t == 0), stop=(t == M - 1))
            coef_sb = sml.tile([2, KL], f32)
            nc.scalar.copy(coef_sb[:], coef_ps[:])
            nc.sync.dma_start(coefd[:].rearrange("(a b) -> a b", a=2), coef_sb[:])
            coef_t = sml.tile([KL, 2], f32)
            nc.sync.dma_start(coef_t[:], coefd[:].rearrange("(two k) -> k two", two=2))

            # ---- phase 8: diag + w ----
            TT = nc.vector.tensor_tensor
            A = mybir.AluOpType
            dd = sml.tile([KL, 1], f32)
            TT(dd[:], sig0p[:], sig0p[:], op=A.mult)
            TT(dd[:], dd[:], emloc[:], op=A.subtract)
            udg = sml.tile([KL, 1], f32)
            nc.vector._custom_dve(SELMAX, out=udg[:], in0=dd[:],
                                  s0=naentL[:], s1=bentL[:], imm2=CLAMP)
            MD = sml.tile([KL, 1], f32)
            nc.scalar.activation(MD[:], udg[:], AF.Derivative_Erf, bias=zero_col[:])

            cwL = sml.tile([KL, 1], f32)
            TT(cwL[:], cospL[:], ssr_p[:], op=A.mult)
            t2 = sml.tile([KL, 1], f32)
            TT(t2[:], MD[:], cwL[:], op=A.mult)
            coef1 = sml.tile([KL, 1], f32)
            TT(coef1[:], coef_t[:, 0:1], t2[:], op=A.subtract)
            nc.vector.tensor_scalar_mul(coef1[:], coef1[:], SQ)
            rsnd = sml.tile([KL, 1], f32)
            TT(rsnd[:], coef_t[:, 1:2], MD[:], op=A.subtract)
            c3 = sml.tile([KL, 1], f32)
            nc.vector.tensor_scalar(c3[:], rsnd[:], -SQ, float(K - 1),
                                    op0=A.mult, op1=A.add)
            ccm = sml.tile([KL, 1], f32)
            TT(ccm[:], MD[:], ssr_p[:], op=A.mult)
            cc = sml.tile([KL, 1], f32)
            TT(cc[:], cchat[:], ccm[:], op=A.subtract)
            nc.vector.tensor_scalar_mul(cc[:], cc[:], SQ)
            coef2 = sml.tile([KL, 1], f32)
            TT(coef2[:], sinpL[:], cc[:], op=A.mult)
            wv = sml.tile([KL, 1], f32)
            TT(wv[:], coef1[:], coef2[:], op=A.add)
            TT(wv[:], wv[:], c3[:], op=A.add)
            wf = sml.tile([KL, 1], f32)
            nc.vector.tensor_scalar_mul(wf[:], wv[:], float(np.pi / 4.0))

            # ---- phase 9: result reduction ----
            res_row = sml.tile([1, S], f32)
            for h in range(2):
                red = pr.tile([1, 2048], f32, tag="red")
                base = 2048 * h
                nv = 2048 if h == 0 else S - 2048
                for (c0, n) in CH[4 * h:4 * h + 4]:
                    nc.tensor.matmul(red[0:1, c0 - base:c0 - base + n], wf[:],
                                     SM[:, c0:c0 + n], start=True, stop=True)
                nc.scalar.copy(res_row[0:1, base:base + nv], red[0:1, 0:nv])
            nc.sync.dma_start(arin[0, 0:S][None, :], res_row[:])

            # ---- phase 10: AllReduce ----
            nc.gpsimd.collective_compute(
                "AllReduce", A.add, replica_groups=rg,
                ins=[arin[:]], outs=[arout[:]])

            # ---- phase 11: final product ----
            res_t = sml.tile([KL, 32], f32)
            env_t = sml.tile([KL, 32], f32)
            nc.sync.dma_start(res_t[:], arout[0, :].rearrange("(p c) -> p c", c=32))
            nc.sync.dma_start(env_t[:], arout[1, :].rearrange("(p c) -> p c", c=32))
            out_t = sml.tile([KL, 32], f32)
            TT(out_t[:], res_t[:], env_t[:], op=A.mult)
            nc.sync.dma_start(out_d[0:4064].rearrange("(p c) -> p c", c=32),
                              out_t[0:127, :])
            nc.sync.dma_start(out_d[4064:S][None, :], out_t[127:128, 0:31])

    nc.compile()
    _CACHE["nc"] = nc
    return nc


def _shard_map(fn, mesh, in_specs, out_specs):
    try:
        return jax.shard_map(fn, mesh=mesh, in_specs=in_specs,
                             out_specs=out_specs, check_vma=False)
    except TypeError:
        return jax.shard_map(fn, mesh=mesh, in_specs=in_specs,
                             out_specs=out_specs, check_rep=False)


def _runner():
    if "runner" in _CACHE:
        return _CACHE["runner"]
    nc = _build()
    from concourse.bass2jax import (install_neuronx_cc_hook, _bass_exec_p,
                                    partition_id_tensor)
    install_neuronx_cc_hook()
    assert nc.dbg_addr is None, "debug build not supported in cached runner"

    partition_name = nc.partition_id_tensor.name if nc.partition_id_tensor else None
    in_names, out_names, out_avals = [], [], []
    for alloc in nc.m.functions[0].allocations:
        if not isinstance(alloc, mybir.MemoryLocationSet):
            continue
        name = alloc.memorylocations[0].name
        if alloc.kind == "ExternalInput":
            if name != partition_name:
                in_names.append(name)
        elif alloc.kind == "ExternalOutput":
            out_avals.append(jax.core.ShapedArray(tuple(alloc.tensor_shape),
                                                  mybir.dt.np(alloc.dtype)))
            out_names.append(name)
    assert in_names == ["packed"] and out_names == ["out"], (in_names, out_names)
    in_names_full = in_names + out_names
    if partition_name is not None:
        in_names_full.append(partition_name)

    def _body(*args):
        operands = list(args)
        if partition_name is not None:
            operands.append(partition_id_tensor())
        outs = _bass_exec_p.bind(
            *operands,
            out_avals=tuple(out_avals),
            in_names=tuple(in_names_full),
            out_names=tuple(out_names),
            lowering_input_output_aliases=(),
            sim_require_finite=True,
            sim_require_nnan=True,
            nc=nc,
        )
        return tuple(outs)

    devices = jax.devices()[:M]
    mesh = Mesh(np.asarray(devices), ("core",))
    P = PartitionSpec
    sharded = jax.jit(
        _shard_map(_body, mesh, (P("core"), P("core")), (P(),)),
        keep_unused=True,
    )
    # out buffers live on device permanently (kernel overwrites every element)
    zeros_dev = jax.device_put(np.zeros(M * S, np.float32),
                               NamedSharding(mesh, P("core")))
    template = np.zeros((M, PB), np.float32)
    _CACHE["runner"] = (sharded, zeros_dev, template)
    return _CACHE["runner"]


def _fill(template, x, smear_window, knot_mean, knot_low, knot_high,
          ent_mean, ent_low, ent_high, polarization):
    f = np.float32
    lo = f(smear_window[0]); up = f(smear_window[1])
    x = np.asarray(x, f)
    km = np.asarray(knot_mean, f)
    kl = np.asarray(knot_low, f); kh = np.asarray(knot_high, f)
    em = np.asarray(ent_mean, f)
    el = np.asarray(ent_low, f); eh = np.asarray(ent_high, f)
    pol = np.asarray(polarization, f)
    r2 = f(1.0 / np.sqrt(2.0))
    aent = np.exp(-el) * r2
    bent = np.exp(-eh) * r2
    cosf = np.cos(pol); sinf = np.sin(pol)

    U = template[:, O_UNIQ:O_UNIQ + N_UNIQ].reshape(M, 14, KL)
    U[:, 0] = (f((up - lo) / SAMPLES / SAMPLES) * x).reshape(M, KL)  # xs
    U[:, 1] = (f(1.0 - lo) * x - km).reshape(M, KL)                # xm
    U[:, 2] = (f((up + lo) / SAMPLES) * x).reshape(M, KL)          # es
    U[:, 3] = (f(-lo) * x).reshape(M, KL)                          # eo (x_iter-free)
    U[:, 4] = (-np.exp(-kl) * r2).reshape(M, KL)                   # nar
    U[:, 5] = (np.exp(-kh) * r2).reshape(M, KL)                    # brr
    U[:, 6] = (-np.exp(-(f(1.0 - lo)) * x) * r2).reshape(M, KL)    # nae
    U[:, 7] = (np.exp(-(f(1.0 + up)) * x) * r2).reshape(M, KL)     # bee
    U[:, 8] = (-aent).reshape(M, KL)                               # naentL
    U[:, 9] = bent.reshape(M, KL)                                  # bentL
    U[:, 10] = em.reshape(M, KL)                                   # emloc
    U[:, 11] = (-em).reshape(M, KL)                                # nemloc
    U[:, 12] = cosf.reshape(M, KL)                                 # cospL
    U[:, 13] = sinf.reshape(M, KL)                                 # sinpL

    shared = np.concatenate([
        np.ascontiguousarray((-aent).reshape(M, KL).T).ravel(),
        np.ascontiguousarray(bent.reshape(M, KL).T).ravel(),
        np.ascontiguousarray(cosf.reshape(M, KL).T).ravel(),
        -em,
    ])
    template[:, O_N8:PB] = shared[None, :]
    return template


def kernel(x, smear_window, knot_mean, knot_low, knot_high,
           ent_mean, ent_low, ent_high, polarization, _trace=False):
    sharded, zeros_dev, template = _runner()
    buf = _fill(template, x, smear_window, knot_mean, knot_low, knot_high,
                ent_mean, ent_low, ent_high, polarization)
    outs = sharded(buf.reshape(-1), zeros_dev)
    _CACHE["last_result"] = None
    return np.asarray(outs[0], np.float32)


# revision 13
# speedup vs baseline: 1.3754x; 1.3754x over previous
"""Trainium2 Bass kernel for nn_KnotEntangle (K=1024, SAMPLES=4096, 8 cores).

Math: the FFT collapses — signal[:,0] = rowsum(smear) and sum_sig = S*smear[:,0].
The [K,K]@[K,S] contraction reduces to result = w @ smear with
w = coef1 + coef2 + c3 built from row/col reductions of the mix matrix
(rank-1 corr => PE-computable deltas). Each gaussian is evaluated as
(sqrt(pi)/2)*Derivative_Erf(u), u = min(max(d*(-a), d*b), CLAMP) via one
custom DVE op; deltas come from rank-2 PE matmuls.

Sharding: knots split 128/core across 8 cores; [K] summaries AllGathered;
[2,4096] partial (result,env) rows AllReduced; output = env*result (each
core holds the full product, so the output is fetched replicated).

Dispatch: the axon tunnel has ~84ms RTT, which dominates wall-clock. The
jitted shard_map callable is built once and cached; per call we upload one
packed input tensor per core, run, and fetch one 16KB replicated output
with a single blocking point so upload+exec+fetch pipeline into ~1 RTT.
"""
import numpy as np

import jax
from jax.sharding import Mesh, PartitionSpec, NamedSharding

import concourse.bacc as bacc
import concourse.tile as tile
import concourse.mybir as mybir

import concourse.dve_ops as dve_ops_mod
from concourse.dve_ops import DveOp, TENSOR_TENSOR_REDUCE
from concourse.dve_spec import Spec, Src0, C0, C1, C2, maxx, minn, lower as dve_lower
from concourse.dve_uop import DveOpSpec

K = 1024
SAMPLES = 4096
S = SAMPLES - 1           # 4095
M = 8                     # cores
KL = K // M               # 128 knots per core
SQ = float(np.sqrt(np.pi) / 2.0)
CLAMP = 30.0
CH = [(512 * i, 512) for i in range(7)] + [(3584, 511)]   # s-chunks

# packed per-core input layout (float32 offsets)
O_UNIQ = 0                 # 14 slots x KL: xs xm es eo nar brr nae bee
N_UNIQ = 14 * KL           #   naentL bentL emloc nemloc cospL sinpL
O_N8 = N_UNIQ              # naent8 [KL,M]
O_B8 = O_N8 + KL * M       # bent8  [KL,M]
O_C8 = O_B8 + KL * M       # cosp8  [KL,M]
O_NEM = O_C8 + KL * M      # nem_full [K]
PB = O_NEM + K             # 5888 floats per core


def _selmax_ref(in0, in1, s0, s1, imm2):
    return np.minimum(np.maximum(in0 * s0, in0 * s1), imm2).astype(np.float32)


def _make_selmax_op():
    name = "KNOT_SELMAX"
    if name in dve_ops_mod._SUB_OPCODE_FOR_NAME:
        return next(op for op in dve_ops_mod.OPS if op.name == name)
    spec = Spec(body=minn(maxx(Src0 * C0, Src0 * C1), C2), reference=_selmax_ref)
    row = dve_ops_mod._CUSTOM_DVE_ROW_BASE + len(dve_ops_mod.OPS)
    assert row < 0x20
    dve_ops_mod._SUB_OPCODE_FOR_NAME[name] = row
    shas = {}
    for ver in ("v3", "v4"):
        uops = dve_lower(spec, ver=ver)
        shas[ver] = DveOpSpec(name=name, opcode=row, uops=uops, rd1_en=False).sha(ver)
    op = DveOp(name, spec, subdim=False, uops_sha=shas)
    dve_ops_mod.OPS.append(op)
    dve_ops_mod.CUSTOM_DVE_SPECS[name] = spec
    return op


_CACHE = {}


def _build():
    if "nc" in _CACHE:
        return _CACHE["nc"]
    SELMAX = _make_selmax_op()
    nc = bacc.Bacc(None, target_bir_lowering=False, num_devices=M)
    f32 = mybir.dt.float32
    AF = mybir.ActivationFunctionType

    packed_d = nc.dram_tensor("packed", [PB], f32, kind="ExternalInput")
    out_d = nc.dram_tensor("out", [S], f32, kind="ExternalOutput")

    agin = nc.dram_tensor("agin", [2 * KL], f32, kind="Internal")
    agout = nc.dram_tensor("agout", [M * 2 * KL], f32, kind="Internal",
                           addr_space="Shared")
    arin = nc.dram_tensor("arin", [2, SAMPLES], f32, kind="Internal")
    arout = nc.dram_tensor("arout", [2, SAMPLES], f32, kind="Internal",
                           addr_space="Shared")
    coefd = nc.dram_tensor("coefd", [2 * KL], f32, kind="Internal")
    rg = [list(range(M))]

    with tile.TileContext(nc) as tc:
        with tc.tile_pool(name="big", bufs=1) as big, \
             tc.tile_pool(name="sml", bufs=1) as sml, \
             tc.tile_pool(name="u", bufs=3) as upool, \
             tc.tile_pool(name="acc", bufs=8) as accp, \
             tc.tile_pool(name="pd", bufs=3, space="PSUM") as pd, \
             tc.tile_pool(name="pr", bufs=1, space="PSUM") as pr, \
             tc.tile_pool(name="pc", bufs=1, space="PSUM") as pc:

            # ---- load constants (slices of the packed input) ----
            # basis row0 = 0..S-1 (exact in f32; 1/SAMPLES folded into xs/es
            # host-side), row1 = ones
            basis = sml.tile([2, S], f32)
            nc.vector.memset(basis[:], 1.0)
            nc.gpsimd.iota(basis[0:1, :], [[1, S]], base=0,
                           channel_multiplier=0,
                           allow_small_or_imprecise_dtypes=True)
            smear_lhsT = sml.tile([2, KL], f32)
            nc.sync.dma_start(smear_lhsT[:],
                              packed_d[0:2 * KL].rearrange("(a b) -> a b", a=2))
            env_lhsT = sml.tile([2, KL], f32)
            nc.sync.dma_start(env_lhsT[:],
                              packed_d[2 * KL:4 * KL].rearrange("(a b) -> a b", a=2))

            def col(slot, name):
                t = sml.tile([KL, 1], f32, tag=f"col_{name}")
                off = slot * KL
                nc.sync.dma_start(t[:], packed_d[off:off + KL][:, None])
                return t

            nar = col(4, "nar"); br = col(5, "brr")
            nae = col(6, "nae"); be = col(7, "bee")
            naentL = col(8, "naentL"); bentL = col(9, "bentL")
            emloc = col(10, "emloc")
            cospL = col(12, "cospL"); sinpL = col(13, "sinpL")
            naent8 = sml.tile([KL, M], f32)
            nc.sync.dma_start(naent8[:],
                              packed_d[O_N8:O_N8 + KL * M]
                              .rearrange("(k m) -> k m", m=M))
            bent8 = sml.tile([KL, M], f32)
            nc.sync.dma_start(bent8[:],
                              packed_d[O_B8:O_B8 + KL * M]
                              .rearrange("(k m) -> k m", m=M))
            cosp8 = sml.tile([KL, M], f32)
            nc.sync.dma_start(cosp8[:],
                              packed_d[O_C8:O_C8 + KL * M]
                              .rearrange("(k m) -> k m", m=M))

            zero_col = sml.tile([KL, 1], f32)
            nc.vector.memset(zero_col[:], 0.0)
            ones_col = sml.tile([KL, 1], f32)
            nc.vector.memset(ones_col[:], 1.0)

            SM = big.tile([KL, S], f32)
            GA = big.tile([KL, S], f32)
            MXa = big.tile([KL, K], f32)
            MXb = big.tile([KL, K], f32)
            ssr_bc = big.tile([KL, K], f32)

            # ---- phase 2: smear ----
            acc8 = accp.tile([KL, M], f32, tag="acc8")
            for ci, (c0, n) in enumerate(CH):
                dl = pd.tile([KL, 512], f32, tag="delta")
                nc.tensor.matmul(dl[:, 0:n], smear_lhsT[:], basis[:, c0:c0 + n],
                                 start=True, stop=True)
                ut = upool.tile([KL, 512], f32, tag="u")
                nc.vector._custom_dve(SELMAX, out=ut[:, 0:n], in0=dl[:, 0:n],
                                      s0=nar[:], s1=br[:], imm2=CLAMP)
                nc.scalar.activation(SM[:, c0:c0 + n], ut[:, 0:n], AF.Derivative_Erf,
                                     bias=zero_col[:], accum_out=acc8[:, ci:ci + 1])
            ssr_p = sml.tile([KL, 1], f32)
            nc.vector.tensor_scalar_mul(ssr_p[:], SM[:, 0:1], float(S) * SQ)
            sig_sum = sml.tile([KL, 1], f32)
            nc.vector.reduce_sum(sig_sum[:], acc8[:], axis=mybir.AxisListType.X)
            sig0p = sml.tile([KL, 1], f32)
            nc.vector.tensor_scalar_mul(sig0p[:], sig_sum[:],
                                        float(SQ / np.sqrt(S)))

            # ---- phase 3: AllGather [sig0p | ssr] ----
            nc.sync.dma_start(agin[0:KL, None], sig0p[:])
            nc.sync.dma_start(agin[KL:2 * KL, None], ssr_p[:])
            nc.gpsimd.collective_compute(
                "AllGather", mybir.AluOpType.bypass, replica_groups=rg,
                ins=[agin[:]], outs=[agout[:]])

            # ---- phase 4: env (overlaps AG) ----
            for (c0, n) in CH:
                dl = pd.tile([KL, 512], f32, tag="delta")
                nc.tensor.matmul(dl[:, 0:n], env_lhsT[:], basis[:, c0:c0 + n],
                                 start=True, stop=True)
                ut = upool.tile([KL, 512], f32, tag="u")
                nc.vector._custom_dve(SELMAX, out=ut[:, 0:n], in0=dl[:, 0:n],
                                      s0=nae[:], s1=be[:], imm2=CLAMP)
                nc.scalar.activation(GA[:, c0:c0 + n], ut[:, 0:n], AF.Derivative_Erf,
                                     bias=zero_col[:])
            # env reduction: two [1,2048] psum halves sharing one pr slot
            env_row = sml.tile([1, S], f32)
            for h in range(2):
                red = pr.tile([1, 2048], f32, tag="red")
                base = 2048 * h
                nv = 2048 if h == 0 else S - 2048
                for (c0, n) in CH[4 * h:4 * h + 4]:
                    nc.tensor.matmul(red[0:1, c0 - base:c0 - base + n], ones_col[:],
                                     GA[:, c0:c0 + n], start=True, stop=True)
                nc.scalar.copy(env_row[0:1, base:base + nv], red[0:1, 0:nv])
            nc.sync.dma_start(arin[1, 0:S][None, :], env_row[:])

            # ---- phase 5: post-AG assembly ----
            rhs_b = sml.tile([2, K], f32)
            nc.vector.memset(rhs_b[:], 1.0)        # row 1 stays all-ones
            mixa_lhsT = sml.tile([2, K], f32)
            ssr8 = sml.tile([KL, M], f32)
            for r in range(M):
                nc.sync.dma_start(rhs_b[0:1, KL * r:KL * (r + 1)],
                                  agout[2 * KL * r:2 * KL * r + KL][None, :])
                nc.sync.dma_start(mixa_lhsT[0:1, KL * r:KL * (r + 1)],
                                  agout[2 * KL * r:2 * KL * r + KL][None, :])
                nc.sync.dma_start(ssr8[:, r:r + 1],
                                  agout[2 * KL * r + KL:2 * KL * (r + 1)][:, None])
                nc.sync.dma_start(
                    ssr_bc[:, KL * r:KL * (r + 1)],
                    agout[2 * KL * r + KL:2 * KL * (r + 1)][None, :]
                    .broadcast_to((KL, KL)))
            nc.sync.dma_start(mixa_lhsT[1:2, :],
                              packed_d[O_NEM:O_NEM + K][None, :])
            rhs_a = sml.tile([2, KL], f32)
            nc.vector.memset(rhs_a[:], 1.0)        # row 1 stays all-ones
            nc.sync.dma_start(rhs_a[0:1, :], agin[0:KL][None, :])
            mixb_lhsT = sml.tile([2, KL], f32)
            nc.sync.dma_start(mixb_lhsT[0:1, :], agin[0:KL][None, :])
            nc.sync.dma_start(mixb_lhsT[1:2, :],
                              packed_d[11 * KL:12 * KL][None, :])
            cw8 = sml.tile([KL, M], f32)
            nc.vector.tensor_tensor(cw8[:], cosp8[:], ssr8[:],
                                    op=mybir.AluOpType.mult)
            wgt = sml.tile([KL, 2 * M], f32)
            nc.vector.memset(wgt[:], 1.0)
            for t in range(M):
                nc.vector.tensor_copy(wgt[:, 2 * t:2 * t + 1], cw8[:, t:t + 1])

            # ---- phase 6: mix block b (cc over global i) ----
            cch = []
            for ci, c0 in enumerate((0, 512)):
                dl = pd.tile([KL, 512], f32, tag="delta")
                nc.tensor.matmul(dl[:], mixb_lhsT[:], rhs_b[:, c0:c0 + 512],
                                 start=True, stop=True)
                ut = upool.tile([KL, 512], f32, tag="u")
                nc.vector._custom_dve(SELMAX, out=ut[:], in0=dl[:],
                                      s0=naentL[:], s1=bentL[:], imm2=CLAMP)
                nc.scalar.activation(MXb[:, c0:c0 + 512], ut[:], AF.Derivative_Erf,
                                     bias=zero_col[:])
                acc = accp.tile([KL, 1], f32, tag="cch")
                trash = upool.tile([KL, 512], f32, tag="u")
                nc.vector._custom_dve(TENSOR_TENSOR_REDUCE, out=trash[:],
                                      in0=MXb[:, c0:c0 + 512],
                                      in1=ssr_bc[:, c0:c0 + 512],
                                      s0=(0.0 if ci == 0 else cch[0][:]), s1=1.0,
                                      accum_out=acc[:])
                cch.append(acc)
            cchat = cch[1]

            # ---- phase 7: mix block a + coef reductions ----
            for t in range(M):
                dl = pd.tile([KL, KL], f32, tag="delta")
                nc.tensor.matmul(dl[:], mixa_lhsT[:, KL * t:KL * (t + 1)], rhs_a[:],
                                 start=True, stop=True)
                ut = upool.tile([KL, KL], f32, tag="u")
                nc.vector._custom_dve(SELMAX, out=ut[:], in0=dl[:],
                                      s0=naent8[:, t:t + 1], s1=bent8[:, t:t + 1],
                                      imm2=CLAMP)
                nc.scalar.activation(MXa[:, KL * t:KL * (t + 1)], ut[:],
                                     AF.Derivative_Erf, bias=zero_col[:])
            coef_ps = pc.tile([2, KL], f32)
            for t in range(M):
                nc.tensor.matmul(coef_ps[:], wgt[:, 2 * t:2 * t + 2],
                                 MXa[:, KL * t:KL * (t + 1)],
                                 start=(t == 0), stop=(t == M - 1))
            coef_sb = sml.tile([2, KL], f32)
            nc.scalar.copy(coef_sb[:], coef_ps[:])
            nc.sync.dma_start(coefd[:].rearrange("(a b) -> a b", a=2), coef_sb[:])
            coef_t = sml.tile([KL, 2], f32)
            nc.sync.dma_start(coef_t[:], coefd[:].rearrange("(two k) -> k two", two=2))

            # ---- phase 8: diag + w ----
            TT = nc.vector.tensor_tensor
            A = mybir.AluOpType
            dd = sml.tile([KL, 1], f32)
            TT(dd[:], sig0p[:], sig0p[:], op=A.mult)
            TT(dd[:], dd[:], emloc[:], op=A.subtract)
            udg = sml.tile([KL, 1], f32)
            nc.vector._custom_dve(SELMAX, out=udg[:], in0=dd[:],
                                  s0=naentL[:], s1=bentL[:], imm2=CLAMP)
            MD = sml.tile([KL, 1], f32)
            nc.scalar.activation(MD[:], udg[:], AF.Derivative_Erf, bias=zero_col[:])

            cwL = sml.tile([KL, 1], f32)
            TT(cwL[:], cospL[:], ssr_p[:], op=A.mult)
            t2 = sml.tile([KL, 1], f32)
            TT(t2[:], MD[:], cwL[:], op=A.mult)
            coef1 = sml.tile([KL, 1], f32)
            TT(coef1[:], coef_t[:, 0:1], t2[:], op=A.subtract)
            nc.vector.tensor_scalar_mul(coef1[:], coef1[:], SQ)
            rsnd = sml.tile([KL, 1], f32)
            TT(rsnd[:], coef_t[:, 1:2], MD[:], op=A.subtract)
            c3 = sml.tile([KL, 1], f32)
            nc.vector.tensor_scalar(c3[:], rsnd[:], -SQ, float(K - 1),
                                    op0=A.mult, op1=A.add)
            ccm = sml.tile([KL, 1], f32)
            TT(ccm[:], MD[:], ssr_p[:], op=A.mult)
            cc = sml.tile([KL, 1], f32)
            TT(cc[:], cchat[:], ccm[:], op=A.subtract)
            nc.vector.tensor_scalar_mul(cc[:], cc[:], SQ)
            coef2 = sml.tile([KL, 1], f32)
            TT(coef2[:], sinpL[:], cc[:], op=A.mult)
            wv = sml.tile([KL, 1], f32)
            TT(wv[:], coef1[:], coef2[:], op=A.add)
            TT(wv[:], wv[:], c3[:], op=A.add)
            wf = sml.tile([KL, 1], f32)
            nc.vector.tensor_scalar_mul(wf[:], wv[:], float(np.pi / 4.0))

            # ---- phase 9: result reduction ----
            res_row = sml.tile([1, S], f32)
            for h in range(2):
                red = pr.tile([1, 2048], f32, tag="red")
                base = 2048 * h
                nv = 2048 if h == 0 else S - 2048
                for (c0, n) in CH[4 * h:4 * h + 4]:
                    nc.tensor.matmul(red[0:1, c0 - base:c0 - base + n], wf[:],
                                     SM[:, c0:c0 + n], start=True, stop=True)
                nc.scalar.copy(res_row[0:1, base:base + nv], red[0:1, 0:nv])
            nc.sync.dma_start(arin[0, 0:S][None, :], res_row[:])

            # ---- phase 10: AllReduce ----
            nc.gpsimd.collective_compute(
                "AllReduce", A.add, replica_groups=rg,
                ins=[arin[:]], outs=[arout[:]])

            # ---- phase 11: final product ----
            res_t = sml.tile([KL, 32], f32)
            env_t = sml.tile([KL, 32], f32)
            nc.sync.dma_start(res_t[:], arout[0, :].rearrange("(p c) -> p c", c=32))
            nc.sync.dma_start(env_t[:], arout[1, :].rearrange("(p c) -> p c", c=32))
            out_t = sml.tile([KL, 32], f32)
            TT(out_t[:], res_t[:], env_t[:], op=A.mult)
            nc.sync.dma_start(out_d[0:4064].rearrange("(p c) -> p c", c=32),
                              out_t[0:127, :])
            nc.sync.dma_start(out_d[4064:S][None, :], out_t[127:128, 0:31])

    nc.compile()
    _CACHE["nc"] = nc
    return nc


def _shard_map(fn, mesh, in_specs, out_specs):
    try:
        return jax.shard_map(fn, mesh=mesh, in_specs=in_specs,
                             out_specs=out_specs, check_vma=False)
    except TypeError:
        return jax.shard_map(fn, mesh=mesh, in_specs=in_specs,
                             out_specs=out_specs, check_rep=False)


def _runner():
    if "runner" in _CACHE:
        return _CACHE["runner"]
    nc = _build()
    from concourse.bass2jax import (install_neuronx_cc_hook, _bass_exec_p,
                                    partition_id_tensor)
    install_neuronx_cc_hook()
    assert nc.dbg_addr is None, "debug build not supported in cached runner"

    partition_name = nc.partition_id_tensor.name if nc.partition_id_tensor else None
    in_names, out_names, out_avals = [], [], []
    for alloc in nc.m.functions[0].allocations:
        if not isinstance(alloc, mybir.MemoryLocationSet):
            continue
        name = alloc.memorylocations[0].name
        if alloc.kind == "ExternalInput":
            if name != partition_name:
                in_names.append(name)
        elif alloc.kind == "ExternalOutput":
            out_avals.append(jax.core.ShapedArray(tuple(alloc.tensor_shape),
                                                  mybir.dt.np(alloc.dtype)))
            out_names.append(name)
    assert in_names == ["packed"] and out_names == ["out"], (in_names, out_names)
    in_names_full = in_names + out_names
    if partition_name is not None:
        in_names_full.append(partition_name)

    def _body(*args):
        operands = list(args)
        if partition_name is not None:
            operands.append(partition_id_tensor())
        outs = _bass_exec_p.bind(
            *operands,
            out_avals=tuple(out_avals),
            in_names=tuple(in_names_full),
            out_names=tuple(out_names),
            lowering_input_output_aliases=(),
            sim_require_finite=True,
            sim_require_nnan=True,
            nc=nc,
        )
        return tuple(outs)

    devices = jax.devices()[:M]
    mesh = Mesh(np.asarray(devices), ("core",))
    P = PartitionSpec
    # out buffers live on device permanently (kernel overwrites every element)
    zeros_dev = jax.device_put(np.zeros(M * S, np.float32),
                               NamedSharding(mesh, P("core")))
    sample = np.zeros(M * PB, np.float32)
    try:
        from concourse.bass2jax import fast_dispatch_compile
        sharded = fast_dispatch_compile(
            lambda: jax.jit(
                _shard_map(_body, mesh, (P("core"), P("core")), (P(),)),
                keep_unused=True,
            ).lower(sample, zeros_dev).compile())
    except Exception:
        sharded = jax.jit(
            _shard_map(_body, mesh, (P("core"), P("core")), (P(),)),
            keep_unused=True,
        )
    template = np.zeros((M, PB), np.float32)
    _CACHE["runner"] = (sharded, zeros_dev, template)
    return _CACHE["runner"]


def _fill(template, x, smear_window, knot_mean, knot_low, knot_high,
          ent_mean, ent_low, ent_high, polarization):
    f = np.float32
    lo = f(smear_window[0]); up = f(smear_window[1])
    x = np.asarray(x, f)
    km = np.asarray(knot_mean, f)
    kl = np.asarray(knot_low, f); kh = np.asarray(knot_high, f)
    em = np.asarray(ent_mean, f)
    el = np.asarray(ent_low, f); eh = np.asarray(ent_high, f)
    pol = np.asarray(polarization, f)
    r2 = f(1.0 / np.sqrt(2.0))
    aent = np.exp(-el) * r2
    bent = np.exp(-eh) * r2
    cosf = np.cos(pol); sinf = np.sin(pol)

    U = template[:, O_UNIQ:O_UNIQ + N_UNIQ].reshape(M, 14, KL)
    U[:, 0] = (f((up - lo) / SAMPLES / SAMPLES) * x).reshape(M, KL)  # xs
    U[:, 1] = (f(1.0 - lo) * x - km).reshape(M, KL)                # xm
    U[:, 2] = (f((up + lo) / SAMPLES) * x).reshape(M, KL)          # es
    U[:, 3] = (f(-lo) * x).reshape(M, KL)                          # eo (x_iter-free)
    U[:, 4] = (-np.exp(-kl) * r2).reshape(M, KL)                   # nar
    U[:, 5] = (np.exp(-kh) * r2).reshape(M, KL)                    # brr
    U[:, 6] = (-np.exp(-(f(1.0 - lo)) * x) * r2).reshape(M, KL)    # nae
    U[:, 7] = (np.exp(-(f(1.0 + up)) * x) * r2).reshape(M, KL)     # bee
    U[:, 8] = (-aent).reshape(M, KL)                               # naentL
    U[:, 9] = bent.reshape(M, KL)                                  # bentL
    U[:, 10] = em.reshape(M, KL)                                   # emloc
    U[:, 11] = (-em).reshape(M, KL)                                # nemloc
    U[:, 12] = cosf.reshape(M, KL)                                 # cospL
    U[:, 13] = sinf.reshape(M, KL)                                 # sinpL

    shared = np.concatenate([
        np.ascontiguousarray((-aent).reshape(M, KL).T).ravel(),
        np.ascontiguousarray(bent.reshape(M, KL).T).ravel(),
        np.ascontiguousarray(cosf.reshape(M, KL).T).ravel(),
        -em,
    ])
    template[:, O_N8:PB] = shared[None, :]
    return template


def kernel(x, smear_window, knot_mean, knot_low, knot_high,
           ent_mean, ent_low, ent_high, polarization, _trace=False):
    sharded, zeros_dev, template = _runner()
    buf = _fill(template, x, smear_window, knot_mean, knot_low, knot_high,
                ent_mean, ent_low, ent_high, polarization)
    outs = sharded(buf.reshape(-1), zeros_dev)
    _CACHE["last_result"] = None
    return np.asarray(outs[0], np.float32)


# revision 16
# speedup vs baseline: 2.0902x; 1.5197x over previous
"""Trainium2 Bass kernel for nn_KnotEntangle (K=1024, SAMPLES=4096, 8 cores).

Math: the FFT collapses — signal[:,0] = rowsum(smear) and sum_sig = S*smear[:,0].
The [K,K]@[K,S] contraction reduces to result = w @ smear with
w = coef1 + coef2 + c3 built from row/col reductions of the mix matrix
(rank-1 corr => PE-computable deltas). Each gaussian is evaluated as
(sqrt(pi)/2)*Derivative_Erf(u), u = min(max(d*(-a), d*b), CLAMP) via one
custom DVE op; deltas come from rank-2 PE matmuls.

Sharding: knots split 128/core across 8 cores; [K] summaries AllGathered;
[2,4096] partial (result,env) rows AllReduced; output = env*result (each
core holds the full product, so the output is fetched replicated).

Dispatch: the axon tunnel has ~84ms RTT, which dominates wall-clock. The
jitted shard_map callable is built once and cached; per call we upload one
packed input tensor per core, run, and fetch one 16KB replicated output
with a single blocking point so upload+exec+fetch pipeline into ~1 RTT.
"""
import numpy as np

import jax
from jax.sharding import Mesh, PartitionSpec, NamedSharding

import concourse.bacc as bacc
import concourse.tile as tile
import concourse.mybir as mybir

import concourse.dve_ops as dve_ops_mod
from concourse.dve_ops import DveOp, TENSOR_TENSOR_REDUCE
from concourse.dve_spec import Spec, Src0, C0, C1, C2, maxx, minn, lower as dve_lower
from concourse.dve_uop import DveOpSpec

K = 1024
SAMPLES = 4096
S = SAMPLES - 1           # 4095
M = 8                     # cores
KL = K // M               # 128 knots per core
SQ = float(np.sqrt(np.pi) / 2.0)
CLAMP = 30.0
CH = [(512 * i, 512) for i in range(7)] + [(3584, 511)]   # s-chunks

# packed per-core input layout (float32 offsets)
O_UNIQ = 0                 # 14 slots x KL: xs xm es eo nar brr nae bee
N_UNIQ = 14 * KL           #   naentL bentL emloc nemloc cospL sinpL
O_N8 = N_UNIQ              # naent8 [KL,M]
O_B8 = O_N8 + KL * M       # bent8  [KL,M]
O_C8 = O_B8 + KL * M       # cosp8  [KL,M]
O_NEM = O_C8 + KL * M      # nem_full [K]
PB = O_NEM + K             # 5888 floats per core


def _selmax_ref(in0, in1, s0, s1, imm2):
    return np.minimum(np.maximum(in0 * s0, in0 * s1), imm2).astype(np.float32)


def _make_selmax_op():
    name = "KNOT_SELMAX"
    if name in dve_ops_mod._SUB_OPCODE_FOR_NAME:
        return next(op for op in dve_ops_mod.OPS if op.name == name)
    spec = Spec(body=minn(maxx(Src0 * C0, Src0 * C1), C2), reference=_selmax_ref)
    row = dve_ops_mod._CUSTOM_DVE_ROW_BASE + len(dve_ops_mod.OPS)
    assert row < 0x20
    dve_ops_mod._SUB_OPCODE_FOR_NAME[name] = row
    shas = {}
    for ver in ("v3", "v4"):
        uops = dve_lower(spec, ver=ver)
        shas[ver] = DveOpSpec(name=name, opcode=row, uops=uops, rd1_en=False).sha(ver)
    op = DveOp(name, spec, subdim=False, uops_sha=shas)
    dve_ops_mod.OPS.append(op)
    dve_ops_mod.CUSTOM_DVE_SPECS[name] = spec
    return op


_CACHE = {}


def _build():
    if "nc" in _CACHE:
        return _CACHE["nc"]
    SELMAX = _make_selmax_op()
    nc = bacc.Bacc(None, target_bir_lowering=False, num_devices=M)
    f32 = mybir.dt.float32
    AF = mybir.ActivationFunctionType

    packed_d = nc.dram_tensor("packed", [PB], f32, kind="ExternalInput")
    out_d = nc.dram_tensor("out", [S], f32, kind="ExternalOutput")

    agin = nc.dram_tensor("agin", [2 * KL], f32, kind="Internal")
    agout = nc.dram_tensor("agout", [M * 2 * KL], f32, kind="Internal",
                           addr_space="Shared")
    # env and result are AllReduced separately: env is ready right after
    # phase 4, so its collective hides behind phases 5-9; the tail collective
    # then carries only the result row (half the payload)
    erin = nc.dram_tensor("erin", [1, SAMPLES], f32, kind="Internal")
    erout = nc.dram_tensor("erout", [1, SAMPLES], f32, kind="Internal",
                           addr_space="Shared")
    arin = nc.dram_tensor("arin", [1, SAMPLES], f32, kind="Internal")
    arout = nc.dram_tensor("arout", [1, SAMPLES], f32, kind="Internal",
                           addr_space="Shared")
    coefd = nc.dram_tensor("coefd", [2 * KL], f32, kind="Internal")
    rg = [list(range(M))]

    with tile.TileContext(nc) as tc:
        with tc.tile_pool(name="big", bufs=1) as big, \
             tc.tile_pool(name="sml", bufs=1) as sml, \
             tc.tile_pool(name="u", bufs=3) as upool, \
             tc.tile_pool(name="acc", bufs=8) as accp, \
             tc.tile_pool(name="pd", bufs=3, space="PSUM") as pd, \
             tc.tile_pool(name="pr", bufs=1, space="PSUM") as pr, \
             tc.tile_pool(name="pc", bufs=1, space="PSUM") as pc:

            # ---- load constants (slices of the packed input) ----
            # basis row0 = 0..S-1 (exact in f32; 1/SAMPLES folded into xs/es
            # host-side), row1 = ones
            basis = sml.tile([2, S], f32)
            nc.vector.memset(basis[:], 1.0)
            nc.gpsimd.iota(basis[0:1, :], [[1, S]], base=0,
                           channel_multiplier=0,
                           allow_small_or_imprecise_dtypes=True)
            smear_lhsT = sml.tile([2, KL], f32)
            nc.sync.dma_start(smear_lhsT[:],
                              packed_d[0:2 * KL].rearrange("(a b) -> a b", a=2))
            env_lhsT = sml.tile([2, KL], f32)
            nc.sync.dma_start(env_lhsT[:],
                              packed_d[2 * KL:4 * KL].rearrange("(a b) -> a b", a=2))

            def col(slot, name):
                t = sml.tile([KL, 1], f32, tag=f"col_{name}")
                off = slot * KL
                nc.sync.dma_start(t[:], packed_d[off:off + KL][:, None])
                return t

            nar = col(4, "nar"); br = col(5, "brr")
            nae = col(6, "nae"); be = col(7, "bee")
            naentL = col(8, "naentL"); bentL = col(9, "bentL")
            emloc = col(10, "emloc")
            cospL = col(12, "cospL"); sinpL = col(13, "sinpL")
            naent8 = sml.tile([KL, M], f32)
            nc.sync.dma_start(naent8[:],
                              packed_d[O_N8:O_N8 + KL * M]
                              .rearrange("(k m) -> k m", m=M))
            bent8 = sml.tile([KL, M], f32)
            nc.sync.dma_start(bent8[:],
                              packed_d[O_B8:O_B8 + KL * M]
                              .rearrange("(k m) -> k m", m=M))
            cosp8 = sml.tile([KL, M], f32)
            nc.sync.dma_start(cosp8[:],
                              packed_d[O_C8:O_C8 + KL * M]
                              .rearrange("(k m) -> k m", m=M))

            zero_col = sml.tile([KL, 1], f32)
            nc.vector.memset(zero_col[:], 0.0)
            ones_col = sml.tile([KL, 1], f32)
            nc.vector.memset(ones_col[:], 1.0)

            SM = big.tile([KL, S], f32)
            GA = big.tile([KL, S], f32)
            MXa = big.tile([KL, K], f32)
            MXb = big.tile([KL, K], f32)
            ssr_bc = big.tile([KL, K], f32)

            # ---- phase 2: smear ----
            acc8 = accp.tile([KL, M], f32, tag="acc8")
            for ci, (c0, n) in enumerate(CH):
                dl = pd.tile([KL, 512], f32, tag="delta")
                nc.tensor.matmul(dl[:, 0:n], smear_lhsT[:], basis[:, c0:c0 + n],
                                 start=True, stop=True)
                ut = upool.tile([KL, 512], f32, tag="u")
                nc.vector._custom_dve(SELMAX, out=ut[:, 0:n], in0=dl[:, 0:n],
                                      s0=nar[:], s1=br[:], imm2=CLAMP)
                nc.scalar.activation(SM[:, c0:c0 + n], ut[:, 0:n], AF.Derivative_Erf,
                                     bias=zero_col[:], accum_out=acc8[:, ci:ci + 1])
            ssr_p = sml.tile([KL, 1], f32)
            nc.vector.tensor_scalar_mul(ssr_p[:], SM[:, 0:1], float(S) * SQ)
            sig_sum = sml.tile([KL, 1], f32)
            nc.vector.reduce_sum(sig_sum[:], acc8[:], axis=mybir.AxisListType.X)
            sig0p = sml.tile([KL, 1], f32)
            nc.vector.tensor_scalar_mul(sig0p[:], sig_sum[:],
                                        float(SQ / np.sqrt(S)))

            # ---- phase 3: AllGather [sig0p | ssr] ----
            nc.sync.dma_start(agin[0:KL, None], sig0p[:])
            nc.sync.dma_start(agin[KL:2 * KL, None], ssr_p[:])
            nc.gpsimd.collective_compute(
                "AllGather", mybir.AluOpType.bypass, replica_groups=rg,
                ins=[agin[:]], outs=[agout[:]])

            # ---- phase 4: env (overlaps AG) ----
            for (c0, n) in CH:
                dl = pd.tile([KL, 512], f32, tag="delta")
                nc.tensor.matmul(dl[:, 0:n], env_lhsT[:], basis[:, c0:c0 + n],
                                 start=True, stop=True)
                ut = upool.tile([KL, 512], f32, tag="u")
                nc.vector._custom_dve(SELMAX, out=ut[:, 0:n], in0=dl[:, 0:n],
                                      s0=nae[:], s1=be[:], imm2=CLAMP)
                nc.scalar.activation(GA[:, c0:c0 + n], ut[:, 0:n], AF.Derivative_Erf,
                                     bias=zero_col[:])
            # env reduction: two [1,2048] psum halves sharing one pr slot
            env_row = sml.tile([1, S], f32)
            for h in range(2):
                red = pr.tile([1, 2048], f32, tag="red")
                base = 2048 * h
                nv = 2048 if h == 0 else S - 2048
                for (c0, n) in CH[4 * h:4 * h + 4]:
                    nc.tensor.matmul(red[0:1, c0 - base:c0 - base + n], ones_col[:],
                                     GA[:, c0:c0 + n], start=True, stop=True)
                nc.scalar.copy(env_row[0:1, base:base + nv], red[0:1, 0:nv])
            nc.sync.dma_start(erin[0, 0:S][None, :], env_row[:])
            nc.gpsimd.collective_compute(
                "AllReduce", mybir.AluOpType.add, replica_groups=rg,
                ins=[erin[:]], outs=[erout[:]])

            # ---- phase 5: post-AG assembly ----
            # issue the big ssr broadcasts first so the 512KB of SBUF writes
            # overlaps the small assembly DMAs below
            for r in range(M):
                nc.sync.dma_start(
                    ssr_bc[:, KL * r:KL * (r + 1)],
                    agout[2 * KL * r + KL:2 * KL * (r + 1)][None, :]
                    .broadcast_to((KL, KL)))
            rhs_b = sml.tile([2, K], f32)
            nc.vector.memset(rhs_b[:], 1.0)        # row 1 stays all-ones
            mixa_lhsT = sml.tile([2, K], f32)
            ssr8 = sml.tile([KL, M], f32)
            for r in range(M):
                nc.sync.dma_start(rhs_b[0:1, KL * r:KL * (r + 1)],
                                  agout[2 * KL * r:2 * KL * r + KL][None, :])
                nc.sync.dma_start(mixa_lhsT[0:1, KL * r:KL * (r + 1)],
                                  agout[2 * KL * r:2 * KL * r + KL][None, :])
                nc.sync.dma_start(ssr8[:, r:r + 1],
                                  agout[2 * KL * r + KL:2 * KL * (r + 1)][:, None])
            nc.sync.dma_start(mixa_lhsT[1:2, :],
                              packed_d[O_NEM:O_NEM + K][None, :])
            rhs_a = sml.tile([2, KL], f32)
            nc.vector.memset(rhs_a[:], 1.0)        # row 1 stays all-ones
            nc.sync.dma_start(rhs_a[0:1, :], agin[0:KL][None, :])
            mixb_lhsT = sml.tile([2, KL], f32)
            nc.sync.dma_start(mixb_lhsT[0:1, :], agin[0:KL][None, :])
            nc.sync.dma_start(mixb_lhsT[1:2, :],
                              packed_d[11 * KL:12 * KL][None, :])
            cw8 = sml.tile([KL, M], f32)
            nc.vector.tensor_tensor(cw8[:], cosp8[:], ssr8[:],
                                    op=mybir.AluOpType.mult)
            wgt = sml.tile([KL, 2 * M], f32)
            nc.vector.memset(wgt[:], 1.0)
            for t in range(M):
                nc.vector.tensor_copy(wgt[:, 2 * t:2 * t + 1], cw8[:, t:t + 1])

            # ---- phase 6: mix block b (cc over global i) ----
            cch = []
            for ci, c0 in enumerate((0, 512)):
                dl = pd.tile([KL, 512], f32, tag="delta")
                nc.tensor.matmul(dl[:], mixb_lhsT[:], rhs_b[:, c0:c0 + 512],
                                 start=True, stop=True)
                ut = upool.tile([KL, 512], f32, tag="u")
                nc.vector._custom_dve(SELMAX, out=ut[:], in0=dl[:],
                                      s0=naentL[:], s1=bentL[:], imm2=CLAMP)
                nc.scalar.activation(MXb[:, c0:c0 + 512], ut[:], AF.Derivative_Erf,
                                     bias=zero_col[:])
                acc = accp.tile([KL, 1], f32, tag="cch")
                trash = upool.tile([KL, 512], f32, tag="u")
                nc.vector._custom_dve(TENSOR_TENSOR_REDUCE, out=trash[:],
                                      in0=MXb[:, c0:c0 + 512],
                                      in1=ssr_bc[:, c0:c0 + 512],
                                      s0=(0.0 if ci == 0 else cch[0][:]), s1=1.0,
                                      accum_out=acc[:])
                cch.append(acc)
            cchat = cch[1]

            # ---- phase 7: mix block a + coef reductions ----
            for t in range(M):
                dl = pd.tile([KL, KL], f32, tag="delta")
                nc.tensor.matmul(dl[:], mixa_lhsT[:, KL * t:KL * (t + 1)], rhs_a[:],
                                 start=True, stop=True)
                ut = upool.tile([KL, KL], f32, tag="u")
                nc.vector._custom_dve(SELMAX, out=ut[:], in0=dl[:],
                                      s0=naent8[:, t:t + 1], s1=bent8[:, t:t + 1],
                                      imm2=CLAMP)
                nc.scalar.activation(MXa[:, KL * t:KL * (t + 1)], ut[:],
                                     AF.Derivative_Erf, bias=zero_col[:])
            coef_ps = pc.tile([2, KL], f32)
            for t in range(M):
                nc.tensor.matmul(coef_ps[:], wgt[:, 2 * t:2 * t + 2],
                                 MXa[:, KL * t:KL * (t + 1)],
                                 start=(t == 0), stop=(t == M - 1))
            coef_sb = sml.tile([2, KL], f32)
            nc.scalar.copy(coef_sb[:], coef_ps[:])
            nc.sync.dma_start(coefd[:].rearrange("(a b) -> a b", a=2), coef_sb[:])
            coef_t = sml.tile([KL, 2], f32)
            nc.sync.dma_start(coef_t[:], coefd[:].rearrange("(two k) -> k two", two=2))

            # ---- phase 8: diag + w ----
            TT = nc.vector.tensor_tensor
            A = mybir.AluOpType
            dd = sml.tile([KL, 1], f32)
            TT(dd[:], sig0p[:], sig0p[:], op=A.mult)
            TT(dd[:], dd[:], emloc[:], op=A.subtract)
            udg = sml.tile([KL, 1], f32)
            nc.vector._custom_dve(SELMAX, out=udg[:], in0=dd[:],
                                  s0=naentL[:], s1=bentL[:], imm2=CLAMP)
            MD = sml.tile([KL, 1], f32)
            nc.scalar.activation(MD[:], udg[:], AF.Derivative_Erf, bias=zero_col[:])

            cwL = sml.tile([KL, 1], f32)
            TT(cwL[:], cospL[:], ssr_p[:], op=A.mult)
            t2 = sml.tile([KL, 1], f32)
            TT(t2[:], MD[:], cwL[:], op=A.mult)
            coef1 = sml.tile([KL, 1], f32)
            TT(coef1[:], coef_t[:, 0:1], t2[:], op=A.subtract)
            nc.vector.tensor_scalar_mul(coef1[:], coef1[:], SQ)
            rsnd = sml.tile([KL, 1], f32)
            TT(rsnd[:], coef_t[:, 1:2], MD[:], op=A.subtract)
            c3 = sml.tile([KL, 1], f32)
            nc.vector.tensor_scalar(c3[:], rsnd[:], -SQ, float(K - 1),
                                    op0=A.mult, op1=A.add)
            ccm = sml.tile([KL, 1], f32)
            TT(ccm[:], MD[:], ssr_p[:], op=A.mult)
            cc = sml.tile([KL, 1], f32)
            TT(cc[:], cchat[:], ccm[:], op=A.subtract)
            nc.vector.tensor_scalar_mul(cc[:], cc[:], SQ)
            coef2 = sml.tile([KL, 1], f32)
            TT(coef2[:], sinpL[:], cc[:], op=A.mult)
            wv = sml.tile([KL, 1], f32)
            TT(wv[:], coef1[:], coef2[:], op=A.add)
            TT(wv[:], wv[:], c3[:], op=A.add)
            wf = sml.tile([KL, 1], f32)
            nc.vector.tensor_scalar_mul(wf[:], wv[:], float(np.pi / 4.0))

            # ---- phase 9: result reduction ----
            res_row = sml.tile([1, S], f32)
            for h in range(2):
                red = pr.tile([1, 2048], f32, tag="red")
                base = 2048 * h
                nv = 2048 if h == 0 else S - 2048
                for (c0, n) in CH[4 * h:4 * h + 4]:
                    nc.tensor.matmul(red[0:1, c0 - base:c0 - base + n], wf[:],
                                     SM[:, c0:c0 + n], start=True, stop=True)
                nc.scalar.copy(res_row[0:1, base:base + nv], red[0:1, 0:nv])
            nc.sync.dma_start(arin[0, 0:S][None, :], res_row[:])

            # ---- phase 10: AllReduce (result only; env already reduced) ----
            nc.gpsimd.collective_compute(
                "AllReduce", A.add, replica_groups=rg,
                ins=[arin[:]], outs=[arout[:]])

            # ---- phase 11: final product ----
            res_t = sml.tile([KL, 32], f32)
            env_t = sml.tile([KL, 32], f32)
            nc.sync.dma_start(res_t[:], arout[0, :].rearrange("(p c) -> p c", c=32))
            nc.sync.dma_start(env_t[:], erout[0, :].rearrange("(p c) -> p c", c=32))
            out_t = sml.tile([KL, 32], f32)
            TT(out_t[:], res_t[:], env_t[:], op=A.mult)
            nc.sync.dma_start(out_d[0:4064].rearrange("(p c) -> p c", c=32),
                              out_t[0:127, :])
            nc.sync.dma_start(out_d[4064:S][None, :], out_t[127:128, 0:31])

    nc.compile()
    _CACHE["nc"] = nc
    return nc


def _shard_map(fn, mesh, in_specs, out_specs):
    try:
        return jax.shard_map(fn, mesh=mesh, in_specs=in_specs,
                             out_specs=out_specs, check_vma=False)
    except TypeError:
        return jax.shard_map(fn, mesh=mesh, in_specs=in_specs,
                             out_specs=out_specs, check_rep=False)


def _runner():
    if "runner" in _CACHE:
        return _CACHE["runner"]
    nc = _build()
    from concourse.bass2jax import (install_neuronx_cc_hook, _bass_exec_p,
                                    partition_id_tensor)
    install_neuronx_cc_hook()
    assert nc.dbg_addr is None, "debug build not supported in cached runner"

    partition_name = nc.partition_id_tensor.name if nc.partition_id_tensor else None
    in_names, out_names, out_avals = [], [], []
    for alloc in nc.m.functions[0].allocations:
        if not isinstance(alloc, mybir.MemoryLocationSet):
            continue
        name = alloc.memorylocations[0].name
        if alloc.kind == "ExternalInput":
            if name != partition_name:
                in_names.append(name)
        elif alloc.kind == "ExternalOutput":
            out_avals.append(jax.core.ShapedArray(tuple(alloc.tensor_shape),
                                                  mybir.dt.np(alloc.dtype)))
            out_names.append(name)
    assert in_names == ["packed"] and out_names == ["out"], (in_names, out_names)
    in_names_full = in_names + out_names
    if partition_name is not None:
        in_names_full.append(partition_name)

    def _body(*args):
        operands = list(args)
        if partition_name is not None:
            operands.append(partition_id_tensor())
        outs = _bass_exec_p.bind(
            *operands,
            out_avals=tuple(out_avals),
            in_names=tuple(in_names_full),
            out_names=tuple(out_names),
            lowering_input_output_aliases=(),
            sim_require_finite=True,
            sim_require_nnan=True,
            nc=nc,
        )
        return tuple(outs)

    devices = jax.devices()[:M]
    mesh = Mesh(np.asarray(devices), ("core",))
    P = PartitionSpec
    # out buffers live on device permanently (kernel overwrites every element)
    zeros_dev = jax.device_put(np.zeros(M * S, np.float32),
                               NamedSharding(mesh, P("core")))
    sample = np.zeros(M * PB, np.float32)
    try:
        from concourse.bass2jax import fast_dispatch_compile
        sharded = fast_dispatch_compile(
            lambda: jax.jit(
                _shard_map(_body, mesh, (P("core"), P("core")), (P(),)),
                keep_unused=True,
            ).lower(sample, zeros_dev).compile())
    except Exception:
        sharded = jax.jit(
            _shard_map(_body, mesh, (P("core"), P("core")), (P(),)),
            keep_unused=True,
        )
    template = np.zeros((M, PB), np.float32)
    _CACHE["runner"] = (sharded, zeros_dev, template)
    return _CACHE["runner"]


def _fill(template, x, smear_window, knot_mean, knot_low, knot_high,
          ent_mean, ent_low, ent_high, polarization):
    f = np.float32
    lo = f(smear_window[0]); up = f(smear_window[1])
    x = np.asarray(x, f)
    km = np.asarray(knot_mean, f)
    kl = np.asarray(knot_low, f); kh = np.asarray(knot_high, f)
    em = np.asarray(ent_mean, f)
    el = np.asarray(ent_low, f); eh = np.asarray(ent_high, f)
    pol = np.asarray(polarization, f)
    r2 = f(1.0 / np.sqrt(2.0))
    aent = np.exp(-el) * r2
    bent = np.exp(-eh) * r2
    cosf = np.cos(pol); sinf = np.sin(pol)

    U = template[:, O_UNIQ:O_UNIQ + N_UNIQ].reshape(M, 14, KL)
    U[:, 0] = (f((up - lo) / SAMPLES / SAMPLES) * x).reshape(M, KL)  # xs
    U[:, 1] = (f(1.0 - lo) * x - km).reshape(M, KL)                # xm
    U[:, 2] = (f((up + lo) / SAMPLES) * x).reshape(M, KL)          # es
    U[:, 3] = (f(-lo) * x).reshape(M, KL)                          # eo (x_iter-free)
    U[:, 4] = (-np.exp(-kl) * r2).reshape(M, KL)                   # nar
    U[:, 5] = (np.exp(-kh) * r2).reshape(M, KL)                    # brr
    U[:, 6] = (-np.exp(-(f(1.0 - lo)) * x) * r2).reshape(M, KL)    # nae
    U[:, 7] = (np.exp(-(f(1.0 + up)) * x) * r2).reshape(M, KL)     # bee
    U[:, 8] = (-aent).reshape(M, KL)                               # naentL
    U[:, 9] = bent.reshape(M, KL)                                  # bentL
    U[:, 10] = em.reshape(M, KL)                                   # emloc
    U[:, 11] = (-em).reshape(M, KL)                                # nemloc
    U[:, 12] = cosf.reshape(M, KL)                                 # cospL
    U[:, 13] = sinf.reshape(M, KL)                                 # sinpL

    shared = np.concatenate([
        np.ascontiguousarray((-aent).reshape(M, KL).T).ravel(),
        np.ascontiguousarray(bent.reshape(M, KL).T).ravel(),
        np.ascontiguousarray(cosf.reshape(M, KL).T).ravel(),
        -em,
    ])
    template[:, O_N8:PB] = shared[None, :]
    return template


def kernel(x, smear_window, knot_mean, knot_low, knot_high,
           ent_mean, ent_low, ent_high, polarization, _trace=False):
    sharded, zeros_dev, template = _runner()
    buf = _fill(template, x, smear_window, knot_mean, knot_low, knot_high,
                ent_mean, ent_low, ent_high, polarization)
    outs = sharded(buf.reshape(-1), zeros_dev)
    _CACHE["last_result"] = None
    return np.asarray(outs[0], np.float32)
